# revision 9
# baseline (speedup 1.0000x reference)
"""Grouped-Query Attention (S=2048, NQ=32, NKV=8, D=128, HID=4096) on 8 TRN2 NeuronCores.

Sharding: tensor-parallel over heads. Core c owns KV head c and its G=4
query heads (rows c*512..(c+1)*512 of Wq, c*128..(c+1)*128 of Wk/Wv, and
columns c*512..(c+1)*512 of Wo).  Each core computes a partial output
(row-parallel Wo); the host sums the 8 partials.

All matmuls run in bf16 (1 cycle/row on PE) with fp32 PSUM accumulation.
Layouts are chosen so no activation transpose is needed except 16 tiny
[128,128] PE transposes of vT:
  - projections produce qT/kT/vT [d, s] (d on partitions)
  - scores are computed transposed: S^T[j,i] = kT.T-slice @ qT  (lhsT=kT)
  - softmax skips max-subtraction (scores are ~N(0, 1.6^2), exp is safe)
  - rowsums come from an all-ones [128,128] stationary matmul which yields
    the denominator already broadcast across all 128 partitions
  - ctx^T[d,i] accumulates with lhsT=v[j,d]; it is exactly the lhsT layout
    the output projection needs.
"""

import os
import sys

import numpy as np
import ml_dtypes

for _p in ("/opt/trn_rl_repo", "/root/.axon_site/_ro/trn_rl_repo"):
    if os.path.isdir(_p) and _p not in sys.path:
        sys.path.insert(0, _p)

import concourse.bass as bass
import concourse.bacc as bacc
import concourse.mybir as mybir
import concourse.tile as tile
from concourse.bass_utils import run_bass_kernel_spmd
from concourse.masks import make_identity

P = 128          # partitions / head dim / PE tile
S = 2048         # sequence length
HID = 4096       # hidden dim
NCORES = 8
NH = 4           # q heads per core
DQ = NH * P      # per-core q width (512)
SC = 512         # free-dim chunk (PSUM bank = 512 fp32)
NKT = HID // P   # 32 contraction tiles over hidden
NCH = S // SC    # 4 sequence chunks
NJT = S // P     # 16 key tiles
NMT = S // P     # 16 out row tiles
NOC = HID // SC  # 8 out column chunks
SCALE = float(P) ** -0.5
BF = mybir.dt.bfloat16
F32 = mybir.dt.float32
BFNP = np.dtype(ml_dtypes.bfloat16)

_CACHE = {}


def _build():
    nc = bacc.Bacc(None, target_bir_lowering=False)
    xT = nc.declare_dram_parameter("xT", [HID, S], BF, isOutput=False)
    WqT = nc.declare_dram_parameter("WqT", [HID, DQ], BF, isOutput=False)
    WkT = nc.declare_dram_parameter("WkT", [HID, P], BF, isOutput=False)
    WvT = nc.declare_dram_parameter("WvT", [HID, P], BF, isOutput=False)
    bvp = nc.declare_dram_parameter("bvp", [P, 1], F32, isOutput=False)
    WoT = nc.declare_dram_parameter("WoT", [DQ, HID], BF, isOutput=False)
    out = nc.declare_dram_parameter("out", [S, HID], F32, isOutput=True)

    with tile.TileContext(nc) as tc:
        with (
            tc.tile_pool(name="consts", bufs=1) as consts,
            tc.tile_pool(name="acts", bufs=1) as acts,
            tc.tile_pool(name="xin", bufs=8) as xin,
            tc.tile_pool(name="epool", bufs=4) as epool,
            tc.tile_pool(name="rpool", bufs=2) as rpool,
            tc.tile_pool(name="opool", bufs=8) as opool,
        ):
            # ---- resident weights / constants ----
            wq = consts.tile([P, NKT, DQ], BF)
            nc.sync.dma_start(out=wq, in_=WqT[:, :].rearrange("(kt p) d -> p kt d", p=P))
            wk = consts.tile([P, NKT, P], BF)
            nc.sync.dma_start(out=wk, in_=WkT[:, :].rearrange("(kt p) d -> p kt d", p=P))
            wv = consts.tile([P, NKT, P], BF)
            nc.sync.dma_start(out=wv, in_=WvT[:, :].rearrange("(kt p) d -> p kt d", p=P))
            wo = consts.tile([P, NH, HID], BF)
            nc.sync.dma_start(out=wo, in_=WoT[:, :].rearrange("(dt p) o -> p dt o", p=P))
            bv_sb = consts.tile([P, 1], F32)
            nc.sync.dma_start(out=bv_sb, in_=bvp[:, :])
            ones_sb = consts.tile([P, P], BF)
            nc.vector.memset(ones_sb, 1.0)
            ident = consts.tile([P, P], BF)
            make_identity(nc, ident)

            # ---- persistent activations (bf16) ----
            qT = acts.tile([P, NH, S], BF)      # per head: [128 d, 2048 s]
            kT = acts.tile([P, S], BF)          # [128 d, 2048 s]
            vT = acts.tile([P, S], BF)          # [128 d, 2048 s]
            v = acts.tile([P, NJT, P], BF)      # [128 j, jt, 128 d]
            ctxT = acts.tile([P, NH, S], BF)    # per head: [128 d, 2048 i]

            # ---- stage A: projections (stream x once) ----
            with tc.tile_pool(name="pacc", bufs=1, space="PSUM") as pacc:
                for c in range(NCH):
                    s0 = c * SC
                    q_ps = [pacc.tile([P, SC], F32, tag="pq%d" % m, name="q_ps%d" % m)
                            for m in range(NH)]
                    k_ps = pacc.tile([P, SC], F32, tag="pk")
                    v_ps = pacc.tile([P, SC], F32, tag="pv")
                    for kt in range(NKT):
                        xt = xin.tile([P, SC], BF)
                        nc.gpsimd.dma_start(out=xt, in_=xT[kt * P:(kt + 1) * P, s0:s0 + SC])
                        st, sp = kt == 0, kt == NKT - 1
                        for m in range(NH):
                            nc.tensor.matmul(q_ps[m], lhsT=wq[:, kt, m * P:(m + 1) * P],
                                             rhs=xt, start=st, stop=sp)
                        nc.tensor.matmul(k_ps, lhsT=wk[:, kt, :], rhs=xt, start=st, stop=sp)
                        nc.tensor.matmul(v_ps, lhsT=wv[:, kt, :], rhs=xt, start=st, stop=sp)
                    for m in range(NH):
                        nc.vector.tensor_copy(out=qT[:, m, s0:s0 + SC], in_=q_ps[m])
                    nc.vector.tensor_copy(out=kT[:, s0:s0 + SC], in_=k_ps)
                    # v = x @ Wv.T + bv  (bias is per-partition in [d, s] layout)
                    nc.scalar.activation(out=vT[:, s0:s0 + SC], in_=v_ps,
                                         func=mybir.ActivationFunctionType.Identity,
                                         bias=bv_sb, scale=1.0)

                # ---- stage A2: v[j, d] via PE transpose of vT ----
                for jt in range(NJT):
                    t_ps = pacc.tile([P, P], BF, tag="ptr", bufs=2)
                    nc.tensor.transpose(t_ps, vT[:, jt * P:(jt + 1) * P], ident)
                    nc.vector.tensor_copy(out=v[:, jt, :], in_=t_ps)

            # ---- stage B: attention per (head, query chunk) ----
            with tc.tile_pool(name="pb", bufs=1, space="PSUM") as pb:
                for h in range(NH):
                    for t in range(NCH):
                        i0 = t * SC
                        ctx_ps = pb.tile([P, SC], F32, tag="pctx", bufs=2)
                        r_ps = pb.tile([P, SC], F32, tag="prow", bufs=2)
                        for jt in range(NJT):
                            s_ps = pb.tile([P, SC], F32, tag="pscore", bufs=3)
                            nc.tensor.matmul(s_ps, lhsT=kT[:, jt * P:(jt + 1) * P],
                                             rhs=qT[:, h, i0:i0 + SC], start=True, stop=True)
                            e_sb = epool.tile([P, SC], BF)
                            nc.scalar.activation(out=e_sb, in_=s_ps,
                                                 func=mybir.ActivationFunctionType.Exp,
                                                 scale=SCALE)
                            st, sp = jt == 0, jt == NJT - 1
                            nc.tensor.matmul(ctx_ps, lhsT=v[:, jt, :], rhs=e_sb,
                                             start=st, stop=sp)
                            nc.tensor.matmul(r_ps, lhsT=ones_sb, rhs=e_sb,
                                             start=st, stop=sp)
                        rec = rpool.tile([P, SC], F32)
                        nc.vector.reciprocal(out=rec, in_=r_ps)
                        nc.vector.tensor_mul(out=ctxT[:, h, i0:i0 + SC],
                                             in0=ctx_ps, in1=rec)

            # ---- stage C: output projection (partial; host sums cores) ----
            with tc.tile_pool(name="pc", bufs=2, space="PSUM") as pc:
                for mt in range(NMT):
                    m0 = mt * P
                    for oc in range(NOC):
                        o0 = oc * SC
                        o_ps = pc.tile([P, SC], F32, tag="pout")
                        for dt_ in range(NH):
                            nc.tensor.matmul(o_ps, lhsT=ctxT[:, dt_, m0:m0 + P],
                                             rhs=wo[:, dt_, o0:o0 + SC],
                                             start=dt_ == 0, stop=dt_ == NH - 1)
                        ob = opool.tile([P, SC], F32)
                        nc.vector.tensor_copy(out=ob, in_=o_ps)
                        nc.gpsimd.dma_start(out=out[m0:m0 + P, o0:o0 + SC], in_=ob)
    nc.finalize()
    return nc


def _get_program():
    if "nc" not in _CACHE:
        _CACHE["nc"] = _build()
    return _CACHE["nc"]


def _prep_inputs(hidden_states, Wq, Wk, Wv, bv, Wo):
    x = np.asarray(hidden_states, np.float32).reshape(S, HID)
    xT = np.ascontiguousarray(x.T).astype(BFNP)
    Wq = np.asarray(Wq, np.float32)
    Wk = np.asarray(Wk, np.float32)
    Wv = np.asarray(Wv, np.float32)
    bv = np.asarray(bv, np.float32)
    Wo = np.asarray(Wo, np.float32)
    maps = []
    for c in range(NCORES):
        qs = slice(c * DQ, (c + 1) * DQ)
        ks = slice(c * P, (c + 1) * P)
        maps.append({
            "xT": xT,
            "WqT": np.ascontiguousarray(Wq[qs].T).astype(BFNP),
            "WkT": np.ascontiguousarray(Wk[ks].T).astype(BFNP),
            "WvT": np.ascontiguousarray(Wv[ks].T).astype(BFNP),
            "bvp": np.ascontiguousarray(bv[ks]).reshape(P, 1),
            "WoT": np.ascontiguousarray(Wo[:, qs].T).astype(BFNP),
        })
    return maps


def kernel(hidden_states, Wq, Wk, Wv, bv, Wo, _trace=False, **kw):
    nc = _get_program()
    maps = _prep_inputs(hidden_states, Wq, Wk, Wv, bv, Wo)
    res = run_bass_kernel_spmd(nc, maps, list(range(NCORES)), trace=_trace, **kw)
    out = np.zeros((S, HID), np.float32)
    for c in range(NCORES):
        out += np.asarray(res.results[c]["out"], np.float32)
    if _trace:
        return out.reshape(1, S, HID), res
    return out.reshape(1, S, HID)
